# revision 15
# baseline (speedup 1.0000x reference)
"""Trainium2 Bass kernel for nn_CrossAttentionLayer.

Computation (per row b of the batch):
  Q = query @ Wq + bq ; K = kv @ Wk + bk ; V = kv @ Wv + bv   (heads H=8, HD=128)
  scores[h,g] = Q[h]·K[g]/sqrt(128); attn = softmax_g; attended[h] = sum_g attn·V[g]
  out = LN(attended @ Wo + bo + query) * gamma + beta

Strategy: pure data parallel over 8 cores (8192 rows each). Per core,
supertiles of 512 rows. Projections run on the TensorEngine in bf16 with
fp32 PSUM accumulation. The per-sample 8x8 head attention is turned into
dense 128x128 matmuls by grouping 16 samples: for a group, rows (s,h)
of a reshaped Q face rows (s,g) of reshaped K; the full 128x128 product
contains the 16 wanted 8x8 blocks on its block diagonal, which a
block-diagonal mask isolates after exp. The resulting block-diagonal
softmax matrix is itself the operand of the attended matmul, so no
per-sample work is ever done on the vector engine.

Layouts:
  XT/YT  [128 d, 8 k, 512 b]  bf16 (DMA-transposed from DRAM bf16 copies)
  QT/KT  [128 d, 8 h, 512 b]  bf16 (matmul: lhsT=W chunk, rhs=XT)
  V16    [128 b, 1024]        bf16 (matmul: lhsT=YT chunk, rhs=Wv)
  VR     [128 (s,g), 8 jj, 128 d]  group reshape of V16 (stream-copy DMA)
  scores psum [128 (s,h), 128 (s,g)] per 16-sample group
  attT   [128 d, 8 jj, 128 (s,h)]  bf16 -> lhsT views for the O projection
"""
import numpy as np
import ml_dtypes
from contextlib import ExitStack

import concourse.bass as bass
import concourse.tile as tile
from concourse import mybir
from concourse.bass_utils import run_bass_kernel_spmd

BF16 = ml_dtypes.bfloat16
F32 = mybir.dt.float32
BF = mybir.dt.bfloat16
AF = mybir.ActivationFunctionType
OP = mybir.AluOpType

N_CORES = 8
B, D, H, HD = 65536, 1024, 8, 128
RPC = B // N_CORES          # rows per core
ST = 512                    # supertile rows
GS = 16                     # samples per attention group (GS*H = 128)
EPS = 1e-5
ISQ = float(1.0 / np.sqrt(HD))


def _build(rows: int) -> bass.Bass:
    nc = bass.Bass("TRN2", target_bir_lowering=False, debug=False,
                   enable_partition_id=False)

    def din(name, shape, dt):
        return nc.dram_tensor(name, shape, dt, kind="ExternalInput").ap()

    query = din("query", [rows, D], F32)
    x16d = din("x16", [rows, D], BF)
    y16d = din("y16", [rows, D], BF)
    wq_d = din("wq", [D, D], BF)
    wk_d = din("wk", [D, D], BF)
    wv_d = din("wv", [D, D], BF)
    wo_d = din("wo", [D, D], BF)
    bq_d = din("bq_dh", [HD, H], F32)
    bk_d = din("bk_dh", [HD, H], F32)
    bvb_d = din("bv_b", [128, D], BF)
    bo_d = din("bo_row", [1, D], F32)
    gam_d = din("gamma_b", [128, D], BF)
    bet_d = din("beta_b", [128, D], F32)
    mask_d = din("mask_bd", [128, 128], BF)
    id_d = din("ident", [128, 128], BF)
    out_d = nc.dram_tensor("out", [rows, D], F32, kind="ExternalOutput").ap()

    n_st = rows // ST

    with tile.TileContext(nc) as tc, ExitStack() as ctx:
        cpool = ctx.enter_context(tc.tile_pool(name="consts", bufs=1))
        wq = cpool.tile([128, 8, D], BF, tag="wq")
        wk = cpool.tile([128, 8, D], BF, tag="wk")
        wv = cpool.tile([128, 8, D], BF, tag="wv")
        wo = cpool.tile([128, 8, D], BF, tag="wo")
        nc.sync.dma_start(wq[:], wq_d.rearrange("(k p) n -> p k n", p=128))
        nc.sync.dma_start(wk[:], wk_d.rearrange("(k p) n -> p k n", p=128))
        nc.sync.dma_start(wv[:], wv_d.rearrange("(k p) n -> p k n", p=128))
        nc.sync.dma_start(wo[:], wo_d.rearrange("(k p) n -> p k n", p=128))
        bq = cpool.tile([HD, H], F32, tag="bq")
        bk = cpool.tile([HD, H], F32, tag="bk")
        bvb = cpool.tile([128, D], BF, tag="bvb")
        bo = cpool.tile([1, D], F32, tag="bo")
        ones1 = cpool.tile([1, 128], F32, tag="ones1")
        gam = cpool.tile([128, D], BF, tag="gam")
        bet = cpool.tile([128, D], F32, tag="bet")
        mask = cpool.tile([128, 128], BF, tag="mask")
        ident = cpool.tile([128, 128], BF, tag="ident")
        nc.sync.dma_start(bq[:], bq_d)
        nc.sync.dma_start(bk[:], bk_d)
        nc.sync.dma_start(bvb[:], bvb_d)
        nc.sync.dma_start(bo[:], bo_d)
        nc.sync.dma_start(gam[:], gam_d)
        nc.sync.dma_start(bet[:], bet_d)
        nc.sync.dma_start(mask[:], mask_d)
        nc.sync.dma_start(ident[:], id_d)
        nc.vector.memset(ones1[:], 1.0)
        epsc = cpool.tile([128, 1], F32, tag="epsc")
        nc.vector.memset(epsc[:], EPS)

        stp = ctx.enter_context(tc.tile_pool(name="stp", bufs=2))
        tp = ctx.enter_context(tc.tile_pool(name="tp", bufs=2))
        qp = ctx.enter_context(tc.tile_pool(name="qtkt", bufs=1))
        scr = ctx.enter_context(tc.tile_pool(name="scr", bufs=1))
        ps = ctx.enter_context(tc.tile_pool(name="ps", bufs=6, space="PSUM"))
        psb = ctx.enter_context(tc.tile_pool(name="psb", bufs=2, space="PSUM"))

        sq_scr = scr.tile([128, 512], F32, tag="sqscr")

        for st in range(n_st):
            s0 = st * ST
            # ---- A) transposed activations ----
            xt = stp.tile([128, 8, ST], BF, tag="xt")
            yt = stp.tile([128, 8, ST], BF, tag="yt")
            for k in range(8):
                nc.sync.dma_start_transpose(
                    xt[:, k, :], x16d[s0:s0 + ST, k * 128:(k + 1) * 128])
                nc.sync.dma_start_transpose(
                    yt[:, k, :], y16d[s0:s0 + ST, k * 128:(k + 1) * 128])

            # ---- B) QT/KT projections (transposed layout) ----
            # [d, b, h] interleaved: for a 16-sample group the (s,h) column
            # block is contiguous -> legal single-free-dim matmul operand.
            qt = qp.tile([128, ST, H], BF, tag="qt")
            kt = qp.tile([128, ST, H], BF, tag="kt")
            for h in range(H):
                p_q = ps.tile([128, 512], F32, tag="ps")
                for k in range(8):
                    nc.tensor.matmul(p_q[:], wq[:, k, h * 128:(h + 1) * 128],
                                     xt[:, k, :], start=(k == 0), stop=(k == 7))
                # bias add + cast to bf16 (ACT, per-partition bias)
                nc.scalar.activation(qt[:, :, h], p_q[:], AF.Identity,
                                     bias=bq[:, h:h + 1])
                p_k = ps.tile([128, 512], F32, tag="ps")
                for k in range(8):
                    nc.tensor.matmul(p_k[:], wk[:, k, h * 128:(h + 1) * 128],
                                     yt[:, k, :], start=(k == 0), stop=(k == 7))
                nc.vector.tensor_scalar_add(kt[:, :, h], p_k[:], bk[:, h:h + 1])

            for t in range(4):
                b0 = t * 128          # tile offset within supertile
                g0 = s0 + b0          # global row offset
                # ---- C) V natural ----
                v16 = tp.tile([128, D], BF, tag="v16")
                for nh in range(2):
                    p_v = ps.tile([128, 512], F32, tag="ps")
                    for k in range(8):
                        nc.tensor.matmul(p_v[:], yt[:, k, b0:b0 + 128],
                                         wv[:, k, nh * 512:(nh + 1) * 512],
                                         start=(k == 0), stop=(k == 7))
                    nc.vector.tensor_tensor(v16[:, nh * 512:(nh + 1) * 512], p_v[:],
                                            bvb[:, nh * 512:(nh + 1) * 512], op=OP.add)
                # ---- D) group reshape of V ----
                vr = tp.tile([128, 8, 128], BF, tag="vr")
                for jj in range(8):
                    nc.sync.dma_start(vr[:, jj, :], v16[jj * GS:(jj + 1) * GS, :])

                # ---- E/F) scores + exp ----
                e16 = tp.tile([128, 8, 128], BF, tag="e16")
                for half in range(2):
                    p_s = ps.tile([128, 512], F32, tag="ps")
                    for q4 in range(4):
                        jj = half * 4 + q4
                        cb = b0 + jj * GS
                        qtv = qt[:, cb:cb + GS, :].rearrange("p s h -> p (s h)")
                        ktv = kt[:, cb:cb + GS, :].rearrange("p s g -> p (s g)")
                        nc.tensor.matmul(p_s[:, q4 * 128:(q4 + 1) * 128], qtv, ktv,
                                         start=True, stop=True)
                    ev = e16[:, half * 4:(half + 1) * 4, :].rearrange("p a b -> p (a b)")
                    nc.scalar.activation(ev, p_s[:], AF.Exp, scale=ISQ)

                # ---- G/H/I/J) mask, rowsum, reciprocal, scale ----
                em = tp.tile([128, 8, 128], BF, tag="em")
                nc.vector.tensor_tensor(em[:], e16[:],
                                        mask[:, None, :].to_broadcast((128, 8, 128)),
                                        op=OP.mult)
                ssum = tp.tile([128, 8], F32, tag="ssum")
                nc.vector.tensor_reduce(ssum[:], em[:], axis=mybir.AxisListType.X,
                                        op=OP.add)
                rs = tp.tile([128, 8], F32, tag="rs")
                nc.vector.reciprocal(rs[:], ssum[:])
                at = tp.tile([128, 8, 128], BF, tag="at")
                for jj in range(8):
                    nc.vector.tensor_scalar_mul(at[:, jj, :], em[:, jj, :],
                                                rs[:, jj:jj + 1])

                # ---- K) transpose attn blocks ----
                atT = tp.tile([128, 8, 128], BF, tag="atT")
                for half in range(2):
                    p_t = psb.tile([128, 512], BF, tag="psb")
                    for q4 in range(4):
                        jj = half * 4 + q4
                        nc.tensor.transpose(p_t[:, q4 * 128:(q4 + 1) * 128],
                                            at[:, jj, :], ident[:])
                    av = atT[:, half * 4:(half + 1) * 4, :].rearrange("p a b -> p (a b)")
                    nc.scalar.copy(av, p_t[:])

                # ---- L) attended ----
                # attT layout [d, h, b]: per-h O-projection lhsT slices are
                # contiguous single-free-dim APs.
                attT = tp.tile([128, H, 128], BF, tag="attT")
                for half in range(2):
                    p_a = ps.tile([128, 512], F32, tag="ps")
                    for q4 in range(4):
                        jj = half * 4 + q4
                        nc.tensor.matmul(p_a[:, q4 * 128:(q4 + 1) * 128],
                                         vr[:, jj, :], atT[:, jj, :],
                                         start=True, stop=True)
                    # psum [128, (4 groups)*(16 s)*(8 h)] -> attT[:, h, jj*16+s]
                    av = (attT[:, :, half * 64:(half + 1) * 64]
                          .rearrange("p h (j s) -> p j s h", j=4))
                    nc.scalar.copy(av, p_a[:].rearrange("p (j s h) -> p j s h",
                                                        j=4, s=GS))

                # ---- M) O projection + residual ----
                qch = tp.tile([128, D], F32, tag="qch")
                nc.sync.dma_start(qch[:], query[g0:g0 + 128, :])
                xres = tp.tile([128, D], F32, tag="xres")
                xs2 = tp.tile([128, 2], F32, tag="xs2")
                sq2 = tp.tile([128, 2], F32, tag="sq2")
                for nh in range(2):
                    p_o = ps.tile([128, 512], F32, tag="ps")
                    nc.tensor.matmul(p_o[:], ones1[:, :],
                                     bo[:, nh * 512:(nh + 1) * 512],
                                     start=True, stop=False)
                    for h in range(H):
                        nc.tensor.matmul(p_o[:], attT[:, h, :],
                                         wo[:, h, nh * 512:(nh + 1) * 512],
                                         start=False, stop=(h == H - 1))
                    nc.vector.scalar_tensor_tensor(
                        xres[:, nh * 512:(nh + 1) * 512], p_o[:], 1.0,
                        qch[:, nh * 512:(nh + 1) * 512], op0=OP.mult, op1=OP.add,
                        accum_out=xs2[:, nh:nh + 1])
                    nc.scalar.activation(sq_scr[:], xres[:, nh * 512:(nh + 1) * 512],
                                         AF.Square, accum_out=sq2[:, nh:nh + 1])

                # ---- N) LN stats ----
                xsum = tp.tile([128, 1], F32, tag="xsum")
                sqs = tp.tile([128, 1], F32, tag="sqs")
                nc.vector.tensor_reduce(xsum[:], xs2[:], axis=mybir.AxisListType.X, op=OP.add)
                nc.vector.tensor_reduce(sqs[:], sq2[:], axis=mybir.AxisListType.X, op=OP.add)
                mu = tp.tile([128, 1], F32, tag="mu")
                ex2 = tp.tile([128, 1], F32, tag="ex2")
                nc.vector.tensor_scalar_mul(mu[:], xsum[:], 1.0 / D)
                nc.vector.tensor_scalar_mul(ex2[:], sqs[:], 1.0 / D)
                mu2 = tp.tile([128, 1], F32, tag="mu2")
                nc.vector.tensor_tensor(mu2[:], mu[:], mu[:], op=OP.mult)
                var = tp.tile([128, 1], F32, tag="var")
                nc.vector.tensor_tensor(var[:], ex2[:], mu2[:], op=OP.subtract)
                lnv = tp.tile([128, 1], F32, tag="lnv")
                nc.scalar.activation(lnv[:], var[:], AF.Ln, bias=epsc[:])
                rstd = tp.tile([128, 1], F32, tag="rstd")
                nc.scalar.activation(rstd[:], lnv[:], AF.Exp, scale=-0.5)
                negc = tp.tile([128, 1], F32, tag="negc")
                nc.vector.tensor_scalar(negc[:], rstd[:], mu[:], -1.0,
                                        op0=OP.mult, op1=OP.mult)

                # ---- O) normalize + affine + store ----
                tn = tp.tile([128, D], F32, tag="tn")
                nc.vector.tensor_scalar(tn[:], xres[:], rstd[:], negc[:],
                                        op0=OP.mult, op1=OP.add)
                tg = tp.tile([128, D], F32, tag="tg")
                nc.vector.tensor_tensor(tg[:], tn[:], gam[:], op=OP.mult)
                osb = tp.tile([128, D], F32, tag="osb")
                nc.vector.tensor_tensor(osb[:], tg[:], bet[:], op=OP.add)
                nc.sync.dma_start(out_d[g0:g0 + 128, :], osb[:])

    return nc


def _split_sync_waits(nc, cap=1):
    """This container's walrus build rejects instructions carrying more
    than `cap` semaphore waits (CoreV3 setupSyncWait: "Too many sync
    wait commands").  Tile's kernel-tail drain aggregates one wait per
    active processor, so redistribute: move leading waits onto fresh
    same-engine Drain instructions inserted just before the offender."""
    import bass_rust
    n_split = 0
    for fn in nc.m.functions:
        for blk in fn.blocks:
            new_insts = []
            changed = False
            for inst in blk.instructions:
                si = inst.sync_info
                waits = list(si.on_wait) if si is not None else []
                if len(waits) > cap:
                    changed = True
                    head, keep = waits[:-cap], waits[-cap:]
                    for i in range(0, len(head), cap):
                        d = bass_rust.InstDrain(
                            name=f"{inst.name}-wsplit{i}", is_reset_sema=False)
                        d.engine = inst.engine
                        d.sync_info = bass_rust.SyncInfo(
                            on_wait=head[i:i + cap], on_update=[])
                        new_insts.append(d)
                        n_split += 1
                    inst.sync_info.on_wait = keep
                new_insts.append(inst)
            if changed:
                blk.instructions = new_insts
    return n_split


_built = {}


def _get_nc(rows: int) -> bass.Bass:
    if rows not in _built:
        nc = _build(rows)
        _split_sync_waits(nc)
        _built[rows] = nc
    return _built[rows]


def make_in_maps(query, key_value, Wq, bq, Wk, bk, Wv, bv, Wo, bo, gamma, beta,
                 rows=RPC, n_cores=N_CORES):
    """CPU-side preprocessing + per-core input maps."""
    x16 = np.ascontiguousarray(query).astype(BF16)
    y16 = np.ascontiguousarray(key_value).astype(BF16)
    consts = {
        "wq": np.ascontiguousarray(Wq).astype(BF16),
        "wk": np.ascontiguousarray(Wk).astype(BF16),
        "wv": np.ascontiguousarray(Wv).astype(BF16),
        "wo": np.ascontiguousarray(Wo).astype(BF16),
        "bq_dh": np.ascontiguousarray(np.asarray(bq, np.float32).reshape(H, HD).T),
        "bk_dh": np.ascontiguousarray(np.asarray(bk, np.float32).reshape(H, HD).T),
        "bv_b": np.broadcast_to(np.asarray(bv).astype(BF16), (128, D)).copy(),
        "bo_row": np.asarray(bo, np.float32).reshape(1, D).copy(),
        "gamma_b": np.broadcast_to(np.asarray(gamma).astype(BF16), (128, D)).copy(),
        "beta_b": np.broadcast_to(np.asarray(beta, np.float32), (128, D)).copy(),
        "mask_bd": np.kron(np.eye(GS, dtype=np.float32),
                           np.ones((H, H), np.float32)).astype(BF16),
        "ident": np.eye(128, dtype=np.float32).astype(BF16),
    }
    q32 = np.ascontiguousarray(np.asarray(query, np.float32))
    in_maps = []
    for c in range(n_cores):
        sl = slice(c * rows, (c + 1) * rows)
        in_maps.append({
            "query": q32[sl],
            "x16": x16[sl],
            "y16": y16[sl],
            **consts,
        })
    return in_maps


_exec_cache = {}


def get_percore_executor(nc):
    """Single-device jitted callable for nc's program (one per core via
    device-resident args). Avoids shard_map: this jax version lowers
    shard_map bodies as a separate HLO computation, which the
    bass_exec neuronx_cc hook rejects."""
    if id(nc) in _exec_cache:
        return _exec_cache[id(nc)]
    import jax
    from concourse.bass2jax import (_bass_exec_p, install_neuronx_cc_hook,
                                    partition_id_tensor)

    install_neuronx_cc_hook()
    partition_name = (nc.partition_id_tensor.name
                      if nc.partition_id_tensor else None)
    in_names, out_names, out_avals = [], [], []
    for alloc in nc.m.functions[0].allocations:
        if not isinstance(alloc, mybir.MemoryLocationSet):
            continue
        name = alloc.memorylocations[0].name
        if alloc.kind == "ExternalInput":
            if name != partition_name:
                in_names.append(name)
        elif alloc.kind == "ExternalOutput":
            out_names.append(name)
            out_avals.append(jax.core.ShapedArray(
                tuple(alloc.tensor_shape), mybir.dt.np(alloc.dtype)))
    all_names = list(in_names) + list(out_names)
    if partition_name is not None:
        all_names.append(partition_name)

    def _body(*args):
        operands = list(args)
        if partition_name is not None:
            operands.append(partition_id_tensor())
        return tuple(_bass_exec_p.bind(
            *operands,
            out_avals=tuple(out_avals),
            in_names=tuple(all_names),
            out_names=tuple(out_names),
            lowering_input_output_aliases=(),
            sim_require_finite=True,
            sim_require_nnan=True,
            nc=nc,
        ))

    fn = jax.jit(_body, keep_unused=True)
    _exec_cache[id(nc)] = (fn, in_names, out_names, out_avals)
    return fn, in_names, out_names, out_avals


def put_core_args(in_map, device, in_names, out_avals):
    import jax
    args = [jax.device_put(np.asarray(in_map[n]), device) for n in in_names]
    args += [jax.device_put(np.zeros(a.shape, a.dtype), device)
             for a in out_avals]
    return args


def _run_percore(nc, in_maps):
    import jax
    fn, in_names, out_names, out_avals = get_percore_executor(nc)
    devices = jax.devices()[:len(in_maps)]
    futs = [fn(*put_core_args(m, d, in_names, out_avals))
            for m, d in zip(in_maps, devices)]
    return [{n: np.asarray(f[i]) for i, n in enumerate(out_names)}
            for f in futs]


def kernel(**inputs) -> np.ndarray:
    in_maps = make_in_maps(**inputs)
    nc = _get_nc(RPC)
    try:
        results = _run_percore(nc, in_maps)
    except Exception:
        results = run_bass_kernel_spmd(nc, in_maps,
                                       list(range(N_CORES))).results
    return np.concatenate([r["out"] for r in results], axis=0)


# revision 17
# speedup vs baseline: 1.4632x; 1.4632x over previous
"""Trainium2 Bass kernel for nn_CrossAttentionLayer.

Computation (per row b of the batch):
  Q = query @ Wq + bq ; K = kv @ Wk + bk ; V = kv @ Wv + bv   (heads H=8, HD=128)
  scores[h,g] = Q[h]·K[g]/sqrt(128); attn = softmax_g; attended[h] = sum_g attn·V[g]
  out = LN(attended @ Wo + bo + query) * gamma + beta

Strategy: pure data parallel over 8 cores (8192 rows each). Per core,
supertiles of 512 rows. Projections run on the TensorEngine in bf16 with
fp32 PSUM accumulation. The per-sample 8x8 head attention is turned into
dense 128x128 matmuls by grouping 16 samples: for a group, rows (s,h)
of a reshaped Q face rows (s,g) of reshaped K; the full 128x128 product
contains the 16 wanted 8x8 blocks on its block diagonal, which a
block-diagonal mask isolates after exp. The resulting block-diagonal
softmax matrix is itself the operand of the attended matmul, so no
per-sample work is ever done on the vector engine.

Layouts:
  XT/YT  [128 d, 8 k, 512 b]  bf16 (DMA-transposed from DRAM bf16 copies)
  QT/KT  [128 d, 8 h, 512 b]  bf16 (matmul: lhsT=W chunk, rhs=XT)
  V16    [128 b, 1024]        bf16 (matmul: lhsT=YT chunk, rhs=Wv)
  VR     [128 (s,g), 8 jj, 128 d]  group reshape of V16 (stream-copy DMA)
  scores psum [128 (s,h), 128 (s,g)] per 16-sample group
  attT   [128 d, 8 jj, 128 (s,h)]  bf16 -> lhsT views for the O projection
"""
import numpy as np
import ml_dtypes
from contextlib import ExitStack

import concourse.bass as bass
import concourse.tile as tile
from concourse import mybir
from concourse.bass_utils import run_bass_kernel_spmd

BF16 = ml_dtypes.bfloat16
F32 = mybir.dt.float32
BF = mybir.dt.bfloat16
AF = mybir.ActivationFunctionType
OP = mybir.AluOpType

N_CORES = 8
B, D, H, HD = 65536, 1024, 8, 128
RPC = B // N_CORES          # rows per core
ST = 512                    # supertile rows
GS = 16                     # samples per attention group (GS*H = 128)
EPS = 1e-5
ISQ = float(1.0 / np.sqrt(HD))


def _build(rows: int, epochs: int = 1) -> bass.Bass:
    nc = bass.Bass("TRN2", target_bir_lowering=False, debug=False,
                   enable_partition_id=False)

    def din(name, shape, dt):
        return nc.dram_tensor(name, shape, dt, kind="ExternalInput").ap()

    query = din("query", [rows, D], F32)
    x16d = din("x16", [rows, D], BF)
    y16d = din("y16", [rows, D], BF)
    wq_d = din("wq", [D, D], BF)
    wk_d = din("wk", [D, D], BF)
    wv_d = din("wv", [D, D], BF)
    wo_d = din("wo", [D, D], BF)
    bq_d = din("bq_dh", [HD, H], F32)
    bk_d = din("bk_dh", [HD, H], F32)
    bvb_d = din("bv_b", [128, D], BF)
    bo_d = din("bo_row", [1, D], F32)
    gam_d = din("gamma_b", [128, D], BF)
    bet_d = din("beta_b", [128, D], F32)
    mask_d = din("mask_bd", [128, 128], BF)
    id_d = din("ident", [128, 128], BF)
    out_d = nc.dram_tensor("out", [rows, D], F32, kind="ExternalOutput").ap()

    n_st = rows // ST

    with tile.TileContext(nc) as tc, ExitStack() as ctx:
        cpool = ctx.enter_context(tc.tile_pool(name="consts", bufs=1))
        wq = cpool.tile([128, 8, D], BF, tag="wq")
        wk = cpool.tile([128, 8, D], BF, tag="wk")
        wv = cpool.tile([128, 8, D], BF, tag="wv")
        wo = cpool.tile([128, 8, D], BF, tag="wo")
        nc.sync.dma_start(wq[:], wq_d.rearrange("(k p) n -> p k n", p=128))
        nc.sync.dma_start(wk[:], wk_d.rearrange("(k p) n -> p k n", p=128))
        nc.sync.dma_start(wv[:], wv_d.rearrange("(k p) n -> p k n", p=128))
        nc.sync.dma_start(wo[:], wo_d.rearrange("(k p) n -> p k n", p=128))
        bq = cpool.tile([HD, H], F32, tag="bq")
        bk = cpool.tile([HD, H], F32, tag="bk")
        bvb = cpool.tile([128, D], BF, tag="bvb")
        bo = cpool.tile([1, D], F32, tag="bo")
        ones1 = cpool.tile([1, 128], F32, tag="ones1")
        gam = cpool.tile([128, D], BF, tag="gam")
        bet = cpool.tile([128, D], F32, tag="bet")
        mask = cpool.tile([128, 128], BF, tag="mask")
        ident = cpool.tile([128, 128], BF, tag="ident")
        nc.sync.dma_start(bq[:], bq_d)
        nc.sync.dma_start(bk[:], bk_d)
        nc.sync.dma_start(bvb[:], bvb_d)
        nc.sync.dma_start(bo[:], bo_d)
        nc.sync.dma_start(gam[:], gam_d)
        nc.sync.dma_start(bet[:], bet_d)
        nc.sync.dma_start(mask[:], mask_d)
        nc.sync.dma_start(ident[:], id_d)
        nc.vector.memset(ones1[:], 1.0)
        epsc = cpool.tile([128, 1], F32, tag="epsc")
        nc.vector.memset(epsc[:], EPS)

        stp = ctx.enter_context(tc.tile_pool(name="stp", bufs=2))
        tp = ctx.enter_context(tc.tile_pool(name="tp", bufs=2))
        qp = ctx.enter_context(tc.tile_pool(name="qtkt", bufs=1))
        scr = ctx.enter_context(tc.tile_pool(name="scr", bufs=1))
        ps = ctx.enter_context(tc.tile_pool(name="ps", bufs=6, space="PSUM"))
        psb = ctx.enter_context(tc.tile_pool(name="psb", bufs=2, space="PSUM"))

        sq_scr = scr.tile([128, 512], F32, tag="sqscr")

        for st in range(n_st * epochs):
            s0 = (st % n_st) * ST
            # ---- A) transposed activations ----
            xt = stp.tile([128, 8, ST], BF, tag="xt")
            yt = stp.tile([128, 8, ST], BF, tag="yt")
            for k in range(8):
                nc.sync.dma_start_transpose(
                    xt[:, k, :], x16d[s0:s0 + ST, k * 128:(k + 1) * 128])
                nc.sync.dma_start_transpose(
                    yt[:, k, :], y16d[s0:s0 + ST, k * 128:(k + 1) * 128])

            # ---- B) QT/KT projections (transposed layout) ----
            # [d, b, h] interleaved: for a 16-sample group the (s,h) column
            # block is contiguous -> legal single-free-dim matmul operand.
            qt = qp.tile([128, ST, H], BF, tag="qt")
            kt = qp.tile([128, ST, H], BF, tag="kt")
            for h in range(H):
                p_q = ps.tile([128, 512], F32, tag="ps")
                for k in range(8):
                    nc.tensor.matmul(p_q[:], wq[:, k, h * 128:(h + 1) * 128],
                                     xt[:, k, :], start=(k == 0), stop=(k == 7))
                # bias add + cast to bf16 (ACT, per-partition bias)
                nc.scalar.activation(qt[:, :, h], p_q[:], AF.Identity,
                                     bias=bq[:, h:h + 1])
                p_k = ps.tile([128, 512], F32, tag="ps")
                for k in range(8):
                    nc.tensor.matmul(p_k[:], wk[:, k, h * 128:(h + 1) * 128],
                                     yt[:, k, :], start=(k == 0), stop=(k == 7))
                nc.vector.tensor_scalar_add(kt[:, :, h], p_k[:], bk[:, h:h + 1])

            for t in range(4):
                b0 = t * 128          # tile offset within supertile
                g0 = s0 + b0          # global row offset
                # ---- C) V natural ----
                v16 = tp.tile([128, D], BF, tag="v16")
                for nh in range(2):
                    p_v = ps.tile([128, 512], F32, tag="ps")
                    for k in range(8):
                        nc.tensor.matmul(p_v[:], yt[:, k, b0:b0 + 128],
                                         wv[:, k, nh * 512:(nh + 1) * 512],
                                         start=(k == 0), stop=(k == 7))
                    nc.vector.tensor_tensor(v16[:, nh * 512:(nh + 1) * 512], p_v[:],
                                            bvb[:, nh * 512:(nh + 1) * 512], op=OP.add)
                # ---- D) group reshape of V ----
                vr = tp.tile([128, 8, 128], BF, tag="vr")
                for jj in range(8):
                    nc.sync.dma_start(vr[:, jj, :], v16[jj * GS:(jj + 1) * GS, :])

                # ---- E/F) scores + exp ----
                e16 = tp.tile([128, 8, 128], BF, tag="e16")
                for half in range(2):
                    p_s = ps.tile([128, 512], F32, tag="ps")
                    for q4 in range(4):
                        jj = half * 4 + q4
                        cb = b0 + jj * GS
                        qtv = qt[:, cb:cb + GS, :].rearrange("p s h -> p (s h)")
                        ktv = kt[:, cb:cb + GS, :].rearrange("p s g -> p (s g)")
                        nc.tensor.matmul(p_s[:, q4 * 128:(q4 + 1) * 128], qtv, ktv,
                                         start=True, stop=True)
                    ev = e16[:, half * 4:(half + 1) * 4, :].rearrange("p a b -> p (a b)")
                    nc.scalar.activation(ev, p_s[:], AF.Exp, scale=ISQ)

                # ---- G/H/I/J) mask, rowsum, reciprocal, scale ----
                em = tp.tile([128, 8, 128], BF, tag="em")
                nc.vector.tensor_tensor(em[:], e16[:],
                                        mask[:, None, :].to_broadcast((128, 8, 128)),
                                        op=OP.mult)
                ssum = tp.tile([128, 8], F32, tag="ssum")
                nc.vector.tensor_reduce(ssum[:], em[:], axis=mybir.AxisListType.X,
                                        op=OP.add)
                rs = tp.tile([128, 8], F32, tag="rs")
                nc.vector.reciprocal(rs[:], ssum[:])
                at = tp.tile([128, 8, 128], BF, tag="at")
                for jj in range(8):
                    nc.vector.tensor_scalar_mul(at[:, jj, :], em[:, jj, :],
                                                rs[:, jj:jj + 1])

                # ---- K) transpose attn blocks ----
                atT = tp.tile([128, 8, 128], BF, tag="atT")
                for half in range(2):
                    p_t = psb.tile([128, 512], BF, tag="psb")
                    for q4 in range(4):
                        jj = half * 4 + q4
                        nc.tensor.transpose(p_t[:, q4 * 128:(q4 + 1) * 128],
                                            at[:, jj, :], ident[:])
                    av = atT[:, half * 4:(half + 1) * 4, :].rearrange("p a b -> p (a b)")
                    nc.scalar.copy(av, p_t[:])

                # ---- L) attended ----
                # attT layout [d, h, b]: per-h O-projection lhsT slices are
                # contiguous single-free-dim APs.
                attT = tp.tile([128, H, 128], BF, tag="attT")
                for half in range(2):
                    p_a = ps.tile([128, 512], F32, tag="ps")
                    for q4 in range(4):
                        jj = half * 4 + q4
                        nc.tensor.matmul(p_a[:, q4 * 128:(q4 + 1) * 128],
                                         vr[:, jj, :], atT[:, jj, :],
                                         start=True, stop=True)
                    # psum [128, (4 groups)*(16 s)*(8 h)] -> attT[:, h, jj*16+s]
                    av = (attT[:, :, half * 64:(half + 1) * 64]
                          .rearrange("p h (j s) -> p j s h", j=4))
                    nc.scalar.copy(av, p_a[:].rearrange("p (j s h) -> p j s h",
                                                        j=4, s=GS))

                # ---- M) O projection + residual ----
                qch = tp.tile([128, D], F32, tag="qch")
                nc.sync.dma_start(qch[:], query[g0:g0 + 128, :])
                xres = tp.tile([128, D], F32, tag="xres")
                xs2 = tp.tile([128, 2], F32, tag="xs2")
                sq2 = tp.tile([128, 2], F32, tag="sq2")
                for nh in range(2):
                    p_o = ps.tile([128, 512], F32, tag="ps")
                    nc.tensor.matmul(p_o[:], ones1[:, :],
                                     bo[:, nh * 512:(nh + 1) * 512],
                                     start=True, stop=False)
                    for h in range(H):
                        nc.tensor.matmul(p_o[:], attT[:, h, :],
                                         wo[:, h, nh * 512:(nh + 1) * 512],
                                         start=False, stop=(h == H - 1))
                    nc.vector.scalar_tensor_tensor(
                        xres[:, nh * 512:(nh + 1) * 512], p_o[:], 1.0,
                        qch[:, nh * 512:(nh + 1) * 512], op0=OP.mult, op1=OP.add,
                        accum_out=xs2[:, nh:nh + 1])
                    nc.scalar.activation(sq_scr[:], xres[:, nh * 512:(nh + 1) * 512],
                                         AF.Square, accum_out=sq2[:, nh:nh + 1])

                # ---- N) LN stats ----
                xsum = tp.tile([128, 1], F32, tag="xsum")
                sqs = tp.tile([128, 1], F32, tag="sqs")
                nc.vector.tensor_reduce(xsum[:], xs2[:], axis=mybir.AxisListType.X, op=OP.add)
                nc.vector.tensor_reduce(sqs[:], sq2[:], axis=mybir.AxisListType.X, op=OP.add)
                mu = tp.tile([128, 1], F32, tag="mu")
                ex2 = tp.tile([128, 1], F32, tag="ex2")
                nc.vector.tensor_scalar_mul(mu[:], xsum[:], 1.0 / D)
                nc.vector.tensor_scalar_mul(ex2[:], sqs[:], 1.0 / D)
                mu2 = tp.tile([128, 1], F32, tag="mu2")
                nc.vector.tensor_tensor(mu2[:], mu[:], mu[:], op=OP.mult)
                var = tp.tile([128, 1], F32, tag="var")
                nc.vector.tensor_tensor(var[:], ex2[:], mu2[:], op=OP.subtract)
                lnv = tp.tile([128, 1], F32, tag="lnv")
                nc.scalar.activation(lnv[:], var[:], AF.Ln, bias=epsc[:])
                rstd = tp.tile([128, 1], F32, tag="rstd")
                nc.scalar.activation(rstd[:], lnv[:], AF.Exp, scale=-0.5)
                negc = tp.tile([128, 1], F32, tag="negc")
                nc.vector.tensor_scalar(negc[:], rstd[:], mu[:], -1.0,
                                        op0=OP.mult, op1=OP.mult)

                # ---- O) normalize + affine + store ----
                tn = tp.tile([128, D], F32, tag="tn")
                nc.vector.tensor_scalar(tn[:], xres[:], rstd[:], negc[:],
                                        op0=OP.mult, op1=OP.add)
                tg = tp.tile([128, D], F32, tag="tg")
                nc.vector.tensor_tensor(tg[:], tn[:], gam[:], op=OP.mult)
                osb = tp.tile([128, D], F32, tag="osb")
                nc.vector.tensor_tensor(osb[:], tg[:], bet[:], op=OP.add)
                nc.sync.dma_start(out_d[g0:g0 + 128, :], osb[:])

    return nc


def _split_sync_waits(nc, cap=1):
    """This container's walrus build rejects instructions carrying more
    than `cap` semaphore waits (CoreV3 setupSyncWait: "Too many sync
    wait commands").  Tile's kernel-tail drain aggregates one wait per
    active processor, so redistribute: move leading waits onto fresh
    same-engine Drain instructions inserted just before the offender."""
    import bass_rust
    n_split = 0
    for fn in nc.m.functions:
        for blk in fn.blocks:
            new_insts = []
            changed = False
            for inst in blk.instructions:
                si = inst.sync_info
                waits = list(si.on_wait) if si is not None else []
                if len(waits) > cap:
                    changed = True
                    head, keep = waits[:-cap], waits[-cap:]
                    for i in range(0, len(head), cap):
                        d = bass_rust.InstDrain(
                            name=f"{inst.name}-wsplit{i}", is_reset_sema=False)
                        d.engine = inst.engine
                        d.sync_info = bass_rust.SyncInfo(
                            on_wait=head[i:i + cap], on_update=[])
                        new_insts.append(d)
                        n_split += 1
                    inst.sync_info.on_wait = keep
                new_insts.append(inst)
            if changed:
                blk.instructions = new_insts
    return n_split


_built = {}


def _get_nc(rows: int) -> bass.Bass:
    if rows not in _built:
        nc = _build(rows)
        _split_sync_waits(nc)
        _built[rows] = nc
    return _built[rows]


def make_in_maps(query, key_value, Wq, bq, Wk, bk, Wv, bv, Wo, bo, gamma, beta,
                 rows=RPC, n_cores=N_CORES):
    """CPU-side preprocessing + per-core input maps."""
    x16 = np.ascontiguousarray(query).astype(BF16)
    y16 = np.ascontiguousarray(key_value).astype(BF16)
    consts = {
        "wq": np.ascontiguousarray(Wq).astype(BF16),
        "wk": np.ascontiguousarray(Wk).astype(BF16),
        "wv": np.ascontiguousarray(Wv).astype(BF16),
        "wo": np.ascontiguousarray(Wo).astype(BF16),
        "bq_dh": np.ascontiguousarray(np.asarray(bq, np.float32).reshape(H, HD).T),
        "bk_dh": np.ascontiguousarray(np.asarray(bk, np.float32).reshape(H, HD).T),
        "bv_b": np.broadcast_to(np.asarray(bv).astype(BF16), (128, D)).copy(),
        "bo_row": np.asarray(bo, np.float32).reshape(1, D).copy(),
        "gamma_b": np.broadcast_to(np.asarray(gamma).astype(BF16), (128, D)).copy(),
        "beta_b": np.broadcast_to(np.asarray(beta, np.float32), (128, D)).copy(),
        "mask_bd": np.kron(np.eye(GS, dtype=np.float32),
                           np.ones((H, H), np.float32)).astype(BF16),
        "ident": np.eye(128, dtype=np.float32).astype(BF16),
    }
    q32 = np.ascontiguousarray(np.asarray(query, np.float32))
    in_maps = []
    for c in range(n_cores):
        sl = slice(c * rows, (c + 1) * rows)
        in_maps.append({
            "query": q32[sl],
            "x16": x16[sl],
            "y16": y16[sl],
            **consts,
        })
    return in_maps


_exec_cache = {}


def get_percore_executor(nc):
    """Single-device jitted callable for nc's program (one per core via
    device-resident args). Avoids shard_map: this jax version lowers
    shard_map bodies as a separate HLO computation, which the
    bass_exec neuronx_cc hook rejects."""
    if id(nc) in _exec_cache:
        return _exec_cache[id(nc)]
    import jax
    from concourse.bass2jax import (_bass_exec_p, install_neuronx_cc_hook,
                                    partition_id_tensor)

    install_neuronx_cc_hook()
    partition_name = (nc.partition_id_tensor.name
                      if nc.partition_id_tensor else None)
    in_names, out_names, out_avals = [], [], []
    for alloc in nc.m.functions[0].allocations:
        if not isinstance(alloc, mybir.MemoryLocationSet):
            continue
        name = alloc.memorylocations[0].name
        if alloc.kind == "ExternalInput":
            if name != partition_name:
                in_names.append(name)
        elif alloc.kind == "ExternalOutput":
            out_names.append(name)
            out_avals.append(jax.core.ShapedArray(
                tuple(alloc.tensor_shape), mybir.dt.np(alloc.dtype)))
    all_names = list(in_names) + list(out_names)
    if partition_name is not None:
        all_names.append(partition_name)

    def _body(*args):
        operands = list(args)
        if partition_name is not None:
            operands.append(partition_id_tensor())
        return tuple(_bass_exec_p.bind(
            *operands,
            out_avals=tuple(out_avals),
            in_names=tuple(all_names),
            out_names=tuple(out_names),
            lowering_input_output_aliases=(),
            sim_require_finite=True,
            sim_require_nnan=True,
            nc=nc,
        ))

    fn = jax.jit(_body, keep_unused=True)
    _exec_cache[id(nc)] = (fn, in_names, out_names, out_avals)
    return fn, in_names, out_names, out_avals


def put_core_args(in_map, device, in_names, out_avals):
    import jax
    args = [jax.device_put(np.asarray(in_map[n]), device) for n in in_names]
    args += [jax.device_put(np.zeros(a.shape, a.dtype), device)
             for a in out_avals]
    return args


def _run_percore(nc, in_maps):
    import jax
    fn, in_names, out_names, out_avals = get_percore_executor(nc)
    devices = jax.devices()[:len(in_maps)]
    futs = [fn(*put_core_args(m, d, in_names, out_avals))
            for m, d in zip(in_maps, devices)]
    return [{n: np.asarray(f[i]) for i, n in enumerate(out_names)}
            for f in futs]


def kernel(**inputs) -> np.ndarray:
    in_maps = make_in_maps(**inputs)
    nc = _get_nc(RPC)
    try:
        results = _run_percore(nc, in_maps)
    except Exception:
        results = run_bass_kernel_spmd(nc, in_maps,
                                       list(range(N_CORES))).results
    return np.concatenate([r["out"] for r in results], axis=0)
